# revision 11
# baseline (speedup 1.0000x reference)
"""DigitCaps dynamic-routing kernel for 8 Trainium2 NeuronCores.

Sharding: J (num_capsule=32) split 8 ways -> 4 capsules per core, batch
replicated. W is SBUF-resident in its natural layout for the i-contraction
GEMMs; the transposed layout is streamed for the p-contraction routing
matmuls. The routing softmax over J uses a cross-core AllReduce of
per-(b,i) partial exp sums; a renormalization-invariance trick keeps a
single running tensor F (= c, up to a shared normalizer) instead of exp(b).

Per core (j = 4 local capsules, B=64, I=2048, Q=16, P=32):
  hat[b,j,i,p] = sum_q x[b,i,q] W[j,i,p,q]       (never materialized)
  v1 = squash(S/32),  S = sum_{i,q} x W          (c1 uniform)
  Delta_k[b,j,i] = sum_q x[b,i,q] * (Wt^T vbd_k)[b,j,(i,q)]
  F <- F * exp(Delta);  Z' = AllReduce_j(sum_j F);  F <- F / Z'   (= c)
  v_k = squash(sum_{i,q} (F x) W)
  out = v3
"""

import numpy as np
import ml_dtypes

import concourse.bacc as bacc
import concourse.mybir as mybir
import concourse.tile as tile
from concourse.bass_utils import run_bass_kernel_spmd
from concourse.masks import make_identity

BF16 = mybir.dt.bfloat16
F32 = mybir.dt.float32
NP_BF16 = ml_dtypes.bfloat16

N_CORES = 8
B = 64
I = 2048
Q = 16
J = 32
P = 32
JL = J // N_CORES
ICH = I // 128
EPS = 1e-7
AF = mybir.ActivationFunctionType

# b-pass half-chunks whose PSUM evac goes through ScalarE (the rest are
# multiplied straight out of PSUM at DVE 1x). Evac unless idx % 8 == 7:
EVAC_MOD = 8
# offload small tree adds to GpSimd (shares an SBUF port with DVE —
# empirical win/loss):
GPSIMD_TREE2 = True
GPSIMD_MERGE = True
GPSIMD_Y = 1  # every Nth... 0=off, else j==JL-1 y-mul on gpsimd

_CACHED = {}


def _squash(nc, small, v_sb, eps_ap):
    """In-place squash over p of v_sb [64, JL*P] fp32 (free = (j, p))."""
    sq = small.tile([B, JL * P], F32, tag="sq")
    nc.vector.tensor_mul(sq[:], v_sb[:], v_sb[:])
    red = sq.rearrange("b (j p) -> b j p", j=JL)
    w = P
    while w > 1:
        h = w // 2
        nc.vector.tensor_add(red[:, :, 0:h], red[:, :, 0:h], red[:, :, h:w])
        w = h
    s2 = small.tile([B, JL], F32, tag="s2")
    nc.vector.tensor_copy(s2[:], red[:, :, 0])
    rt = small.tile([B, JL], F32, tag="rt")
    nc.scalar.activation(rt[:], s2[:], AF.Sqrt, bias=eps_ap[:B, :])
    den = small.tile([B, JL], F32, tag="den")
    nc.vector.tensor_mul(den[:], s2[:], rt[:])
    nc.vector.tensor_add(den[:], den[:], rt[:])
    rec = small.tile([B, JL], F32, tag="rec")
    nc.vector.reciprocal(rec[:], den[:])
    scale = small.tile([B, JL], F32, tag="scale")
    nc.vector.tensor_mul(scale[:], s2[:], rec[:])
    vv = v_sb.rearrange("b (j p) -> b j p", j=JL)
    sc_b = scale.unsqueeze(2).broadcast_to([B, JL, P])
    nc.vector.tensor_mul(vv[:], vv[:], sc_b[:])


def _build_vbd(nc, small, psum_t, v_sb, identity):
    """v_sb [64, (j,p)] fp32 -> two block-diag bf16 lhsT [128, (jj 2, b 64)]."""
    vt_ps = psum_t.tile([128, B], F32, tag="vt_ps")
    nc.tensor.transpose(vt_ps[:], v_sb[:], identity[:B, :B])
    vt = small.tile([128, B], F32, tag="vt")
    nc.scalar.copy(vt[:], vt_ps[:])  # [(j,p), b]
    vbds = []
    for pair in range(2):
        vbd = small.tile([128, 2 * B], BF16, tag=f"vbd{pair}")
        nc.vector.memset(vbd[:], 0.0)
        for jj in range(2):
            j = pair * 2 + jj
            nc.vector.tensor_copy(
                vbd[j * P:(j + 1) * P, jj * B:(jj + 1) * B],
                vt[j * P:(j + 1) * P, :],
            )
        vbds.append(vbd)
    return vbds


def _vT_to_v(nc, small, ps_vt, vT_ps, identity, scale=None):
    """vT psum [128 (j,p), 64 b] -> v_sb [64, (j,p)] fp32 via evac+transpose."""
    vT = small.tile([128, B], F32, tag="vTe")
    if scale is None:
        nc.scalar.copy(vT[:], vT_ps[:])
    else:
        nc.scalar.mul(vT[:], vT_ps[:], scale)
    v_ps = ps_vt.tile([B, 128], F32, tag="v_ps2")
    nc.tensor.transpose(v_ps[:], vT[:], identity[:])
    v_sb = small.tile([B, JL * P], F32, tag="v")
    nc.scalar.copy(v_sb[:], v_ps[:])
    return v_sb


def build_kernel():
    if "nc" in _CACHED:
        return _CACHED["nc"]
    nc = bacc.Bacc(
        "TRN2", target_bir_lowering=False, debug=False, num_devices=N_CORES
    )
    wn_d = nc.dram_tensor("wn", [128, ICH * Q * JL * P], BF16, kind="ExternalInput")
    wt_d = nc.dram_tensor("wt", [128, I * Q], BF16, kind="ExternalInput")
    xq_d = nc.dram_tensor("xq", [128, ICH * Q * B], BF16, kind="ExternalInput")
    xt_d = nc.dram_tensor("xt", [128, I * Q], BF16, kind="ExternalInput")
    out_d = nc.dram_tensor("o", [B, JL * P], F32, kind="ExternalOutput")

    with tile.TileContext(nc) as tc:
        with (
            tc.tile_pool(name="big", bufs=1) as big,
            tc.tile_pool(name="wts", bufs=3) as wts,
            tc.tile_pool(name="tstr", bufs=4) as tstr,
            tc.tile_pool(name="estr", bufs=2) as estr,
            tc.tile_pool(name="small", bufs=1) as small,
            tc.tile_pool(name="ytile", bufs=5) as ytile,
            tc.tile_pool(name="dram", bufs=4, space="DRAM") as dram,
        ):
            # ---- resident loads (chunked so consumers start early) ----
            wn = big.tile([128, ICH * Q * JL * P], BF16, tag="wn")   # 64K/part
            wn_chunk = ICH * Q * JL * P // 4
            for c in range(4):
                nc.sync.dma_start(
                    wn[:, c * wn_chunk:(c + 1) * wn_chunk],
                    wn_d[:, c * wn_chunk:(c + 1) * wn_chunk],
                )
            wnv = wn.rearrange("k (ich q j p) -> k ich q j p", ich=ICH, q=Q, j=JL)
            xq = big.tile([128, ICH * Q * B], BF16, tag="xq")        # 32K/part
            xq_chunk = ICH * Q * B // 2
            for c in range(2):
                nc.sync.dma_start(
                    xq[:, c * xq_chunk:(c + 1) * xq_chunk],
                    xq_d[:, c * xq_chunk:(c + 1) * xq_chunk],
                )
            xqv = xq.rearrange("k (ich q b) -> k ich q b", ich=ICH, q=Q)
            xt = big.tile([128, I * Q], BF16, tag="xt")              # 64K/part

            identity = big.tile([128, 128], F32, tag="ident")
            make_identity(nc, identity[:])
            identb = big.tile([128, 128], BF16, tag="identb")
            make_identity(nc, identb[:])
            eps_t = big.tile([128, 1], F32, tag="eps")
            nc.vector.memset(eps_t[:], EPS)

            # F[ip, (ich, j, b)] bf16: running c (up to global normalizer)
            f_sb = big.tile([128, ICH * JL * B], BF16, tag="f")      # 8K/part
            f_v = f_sb.rearrange("k (ich j b) -> k ich j b", ich=ICH, j=JL)

            # warmup collective to absorb core-start skew
            wu_s = small.tile([128, 8], F32, tag="wu")
            nc.gpsimd.memset(wu_s[:], 0.0)
            wu_i = dram.tile([128, 8], F32, tag="wu_i")
            wu_o = dram.tile([128, 8], F32, tag="wu_o")
            nc.gpsimd.dma_start(wu_i[:], wu_s[:])
            nc.gpsimd.collective_compute(
                "AllReduce", mybir.AluOpType.add,
                replica_groups=[list(range(N_CORES))],
                ins=[wu_i.opt()], outs=[wu_o.opt()],
            )

            # ---- S-pass: vT[(j,p), b] = sum_{i,q} W x ---------------
            with tc.tile_pool(name="ps_s", bufs=1, space="PSUM") as ps_s, \
                 tc.tile_pool(name="ps_st", bufs=1, space="PSUM") as ps_st:
                s_ps = ps_s.tile([128, B], F32, tag="s_ps")
                n_mm = ICH * Q
                k = 0
                for ich in range(ICH):
                    for q in range(Q):
                        nc.tensor.matmul(
                            s_ps[:],
                            wnv[:, ich, q, :, :],       # lhsT [128, (j p)]
                            xqv[:, ich, q, :],          # rhs  [128, 64]
                            start=(k == 0), stop=(k == n_mm - 1),
                        )
                        k += 1
                v_sb = _vT_to_v(nc, small, ps_st, s_ps, identity, scale=1.0 / J)
                _squash(nc, small, v_sb, eps_t)
                vbds = _build_vbd(nc, small, ps_st, v_sb, identity)

            # xt load issued after the S-pass so wn/xq own the early DMA bw
            xt_chunk = I * Q // 4
            for c in range(4):
                nc.sync.dma_start(
                    xt[:, c * xt_chunk:(c + 1) * xt_chunk],
                    xt_d[:, c * xt_chunk:(c + 1) * xt_chunk],
                )

            # ---- 2 routing iterations -------------------------------
            for it in range(2):
                first = it == 0
                # b-pass: Delta[b,j,i] via t = vbd^T Wt, u = t*x, tree over q
                cc_pend = [None, None]
                with tc.tile_pool(name=f"ps_b{it}", bufs=3, space="PSUM") as ps_b, \
                     tc.tile_pool(name=f"ps_bt{it}", bufs=2, space="PSUM") as ps_bt:
                    for g in range(ICH):
                        wt_s = wts.tile([128, 128 * Q], BF16, tag="wt_s")
                        nc.sync.dma_start(
                            wt_s[:], wt_d[:, g * 128 * Q:(g + 1) * 128 * Q]
                        )
                        xoff = g * 2048
                        for pair in range(2):
                            d_ps = ps_bt.tile(
                                [128, 128], BF16, tag="d_ps",
                                name=f"d_ps{it}_{g}_{pair}",
                            )
                            tss = []
                            for half in range(2):
                                t_ps = ps_b.tile(
                                    [128, 1024], F32, tag="t_ps",
                                    name=f"t_ps{it}_{g}_{pair}_{half}",
                                )
                                for m in range(2):
                                    off = half * 1024 + m * 512
                                    nc.tensor.matmul(
                                        t_ps[:, m * 512:(m + 1) * 512],
                                        vbds[pair][:],
                                        wt_s[:, off:off + 512],
                                        start=True, stop=True,
                                    )
                                ts = tstr.tile(
                                    [128, 1024], BF16, tag="ts",
                                    name=f"ts{it}_{g}_{pair}_{half}",
                                )
                                xsl = xt[:, xoff + half * 1024:
                                         xoff + half * 1024 + 1024]
                                hidx = g * 4 + pair * 2 + half
                                if hidx % EVAC_MOD != EVAC_MOD - 1:
                                    nc.scalar.copy(ts[:], t_ps[:])
                                    nc.vector.tensor_mul(ts[:], ts[:], xsl)
                                else:
                                    nc.vector.tensor_mul(ts[:], t_ps[:], xsl)
                                # tree over q within the half: 1024->512->256
                                nc.vector.tensor_add(
                                    ts[:, 0:512], ts[:, 0:512], ts[:, 512:1024]
                                )
                                eng2 = nc.gpsimd if GPSIMD_TREE2 else nc.vector
                                eng2.tensor_add(
                                    ts[:, 0:256], ts[:, 0:256], ts[:, 256:512]
                                )
                                tss.append(ts)
                            engm = nc.gpsimd if GPSIMD_MERGE else nc.vector
                            engm.tensor_add(
                                tss[0][:, 0:256], tss[0][:, 0:256],
                                tss[1][:, 0:256],
                            )
                            nc.vector.tensor_add(
                                tss[0][:, 0:128], tss[0][:, 0:128],
                                tss[0][:, 128:256],
                            )
                            nc.tensor.transpose(
                                d_ps[:], tss[0][:, 0:128], identb[:]
                            )
                            off = (g * JL + pair * 2) * B
                            dst = f_sb[:, off:off + 2 * B]
                            if first:
                                nc.scalar.activation(dst, d_ps[:], AF.Exp)
                            else:
                                ex = estr.tile(
                                    [128, 128], BF16, tag="ex",
                                    name=f"ex{it}_{g}_{pair}",
                                )
                                nc.scalar.activation(ex[:], d_ps[:], AF.Exp)
                                nc.vector.tensor_mul(dst, dst, ex[:])
                        if g == 7 or g == ICH - 1:
                            h = 0 if g == 7 else 1
                            sl = slice(h * 8, h * 8 + 8)
                            zph = small.tile(
                                [128, 8 * B], BF16, tag=f"zp{h}",
                                name=f"zp{it}_{h}",
                            )
                            zpv = zph.rearrange("k (ic b) -> k ic b", ic=8)
                            nc.vector.tensor_add(
                                zpv[:], f_v[:, sl, 0, :], f_v[:, sl, 1, :]
                            )
                            for j in range(2, JL):
                                nc.vector.tensor_add(
                                    zpv[:], zpv[:], f_v[:, sl, j, :]
                                )
                            cc_i = dram.tile(
                                [128, 8 * B], BF16, tag=f"cc_i{h}",
                                name=f"cci{it}_{h}",
                            )
                            cc_o = dram.tile(
                                [128, 8 * B], BF16, tag=f"cc_o{h}",
                                name=f"cco{it}_{h}",
                            )
                            nc.gpsimd.dma_start(cc_i[:], zph[:])
                            nc.gpsimd.collective_compute(
                                "AllReduce", mybir.AluOpType.add,
                                replica_groups=[list(range(N_CORES))],
                                ins=[cc_i.opt()], outs=[cc_o.opt()],
                            )
                            cc_pend[h] = cc_o

                # v-pass: vT[(j,p), b] = sum_{i,q} W (F x), col-tiled over j
                with tc.tile_pool(name=f"ps_v{it}", bufs=1, space="PSUM") as ps_v, \
                     tc.tile_pool(name=f"ps_vt{it}", bufs=2, space="PSUM") as ps_vt:
                    vT_ps = ps_v.tile([128, B], F32, tag="vT_ps")
                    for h in range(2):
                        sl = slice(h * 8, h * 8 + 8)
                        zh = small.tile(
                            [128, 8 * B], BF16, tag=f"z{h}", name=f"z{it}_{h}"
                        )
                        nc.sync.dma_start(zh[:], cc_pend[h][:])
                        with nc.allow_low_precision(
                            reason="softmax normalizer; tol 2e-2"
                        ):
                            nc.vector.reciprocal(zh[:], zh[:])
                        zrv = zh.rearrange("k (ic b) -> k ic b", ic=8)
                        for j in range(JL):
                            nc.vector.tensor_mul(
                                f_v[:, sl, j, :], f_v[:, sl, j, :], zrv[:]
                            )
                    for ich in range(ICH):
                        ys = []
                        for j in range(JL):
                            y = ytile.tile(
                                [128, Q * B], BF16, tag="y",
                                name=f"y{it}_{ich}_{j}",
                            )
                            yv = y.rearrange("k (q b) -> k q b", q=Q)
                            cb = (
                                f_v[:, ich, j, :]
                                .unsqueeze(1).broadcast_to([128, Q, B])
                            )
                            yeng = (
                                nc.gpsimd
                                if (GPSIMD_Y and j == JL - 1) else nc.vector
                            )
                            yeng.tensor_mul(
                                yv[:], xqv[:, ich, :, :], cb[:]
                            )
                            ys.append(yv)
                        for q in range(Q):
                            for j in range(JL):
                                nc.tensor.matmul(
                                    vT_ps[j * P:(j + 1) * P, :],
                                    wnv[:, ich, q, j, :],
                                    ys[j][:, q, :],
                                    start=(ich == 0 and q == 0),
                                    stop=(ich == ICH - 1 and q == Q - 1),
                                    tile_position=(0, j * P),
                                )
                    v_sb = _vT_to_v(nc, small, ps_vt, vT_ps, identity)
                    _squash(nc, small, v_sb, eps_t)
                    if it == 0:
                        vbds = _build_vbd(nc, small, ps_vt, v_sb, identity)
                    else:
                        nc.sync.dma_start(out_d[:], v_sb[:])

    nc.compile()
    _CACHED["nc"] = nc
    return nc


def _prep_inputs(inputs_np, W_np):
    x = np.ascontiguousarray(inputs_np)           # [B, I, Q] f32
    W = np.ascontiguousarray(W_np)                # [J, I, P, Q] f32
    xq = (
        x.reshape(B, ICH, 128, Q).transpose(2, 1, 3, 0)
        .astype(NP_BF16).reshape(128, ICH * Q * B)
    )
    # xt cols ordered (g, q, iw): matches wt streaming windows
    xt_base = (
        x.reshape(B, ICH, 128, Q).transpose(0, 1, 3, 2)   # [b, g, q, iw]
        .astype(NP_BF16).reshape(B, I * Q)
    )
    xt = np.concatenate([xt_base, xt_base], axis=0)
    in_maps = []
    for r in range(N_CORES):
        Wr = W[r * JL:(r + 1) * JL]                       # [4, I, P, Q]
        wn = (
            Wr.reshape(JL, ICH, 128, P, Q).transpose(2, 1, 4, 0, 3)
            .astype(NP_BF16).reshape(128, ICH * Q * JL * P)
        )
        wt = (
            Wr.reshape(JL, ICH, 128, P, Q)
            .transpose(0, 3, 1, 4, 2)                     # [j, p, g, q, iw]
            .astype(NP_BF16).reshape(128, I * Q)
        )
        in_maps.append(
            {
                "wn": np.ascontiguousarray(wn),
                "wt": np.ascontiguousarray(wt),
                "xq": np.ascontiguousarray(xq),
                "xt": np.ascontiguousarray(xt),
            }
        )
    return in_maps


def kernel(inputs, W, _trace=False):
    nc = build_kernel()
    in_maps = _prep_inputs(np.asarray(inputs), np.asarray(W))
    res = run_bass_kernel_spmd(nc, in_maps, list(range(N_CORES)), trace=_trace)
    out = np.concatenate(
        [res.results[r]["o"].reshape(B, JL, P) for r in range(N_CORES)], axis=1
    )
    if _trace:
        kernel.last_exec_ns = res.exec_time_ns
        kernel.last_results = res
    return out.astype(np.float32)


# revision 12
# speedup vs baseline: 1.3520x; 1.3520x over previous
"""DigitCaps dynamic-routing kernel for 8 Trainium2 NeuronCores.

Sharding: J (num_capsule=32) split 8 ways -> 4 capsules per core, batch
replicated. W is SBUF-resident in its natural layout for the i-contraction
GEMMs; the transposed layout is streamed for the p-contraction routing
matmuls. The routing softmax over J uses a cross-core AllReduce of
per-(b,i) partial exp sums; a renormalization-invariance trick keeps a
single running tensor F (= c, up to a shared normalizer) instead of exp(b).

Per core (j = 4 local capsules, B=64, I=2048, Q=16, P=32):
  hat[b,j,i,p] = sum_q x[b,i,q] W[j,i,p,q]       (never materialized)
  v1 = squash(S/32),  S = sum_{i,q} x W          (c1 uniform)
  Delta_k[b,j,i] = sum_q x[b,i,q] * (Wt^T vbd_k)[b,j,(i,q)]
  F <- F * exp(Delta);  Z' = AllReduce_j(sum_j F);  F <- F / Z'   (= c)
  v_k = squash(sum_{i,q} (F x) W)
  out = v3
"""

import numpy as np
import ml_dtypes

import concourse.bacc as bacc
import concourse.mybir as mybir
import concourse.tile as tile
from concourse.bass_utils import run_bass_kernel_spmd
from concourse.masks import make_identity

BF16 = mybir.dt.bfloat16
F32 = mybir.dt.float32
NP_BF16 = ml_dtypes.bfloat16

N_CORES = 8
B = 64
I = 2048
Q = 16
J = 32
P = 32
JL = J // N_CORES
ICH = I // 128
EPS = 1e-7
AF = mybir.ActivationFunctionType

# b-pass half-chunks whose PSUM evac goes through ScalarE (the rest are
# multiplied straight out of PSUM at DVE 1x). Evac unless idx % 8 == 7:
EVAC_MOD = 8
# offload small tree adds to GpSimd (shares an SBUF port with DVE —
# empirical win/loss):
GPSIMD_TREE2 = False
GPSIMD_MERGE = False
GPSIMD_Y = 0  # gpsimd elementwise measured 3x slower + slows DVE via
# shared-SBUF-port contention; keep off

_CACHED = {}


def _squash(nc, small, v_sb, eps_ap):
    """In-place squash over p of v_sb [64, JL*P] fp32 (free = (j, p))."""
    sq = small.tile([B, JL * P], F32, tag="sq")
    nc.vector.tensor_mul(sq[:], v_sb[:], v_sb[:])
    red = sq.rearrange("b (j p) -> b j p", j=JL)
    w = P
    while w > 1:
        h = w // 2
        nc.vector.tensor_add(red[:, :, 0:h], red[:, :, 0:h], red[:, :, h:w])
        w = h
    s2 = small.tile([B, JL], F32, tag="s2")
    nc.vector.tensor_copy(s2[:], red[:, :, 0])
    rt = small.tile([B, JL], F32, tag="rt")
    nc.scalar.activation(rt[:], s2[:], AF.Sqrt, bias=eps_ap[:B, :])
    den = small.tile([B, JL], F32, tag="den")
    nc.vector.tensor_mul(den[:], s2[:], rt[:])
    nc.vector.tensor_add(den[:], den[:], rt[:])
    rec = small.tile([B, JL], F32, tag="rec")
    nc.vector.reciprocal(rec[:], den[:])
    scale = small.tile([B, JL], F32, tag="scale")
    nc.vector.tensor_mul(scale[:], s2[:], rec[:])
    vv = v_sb.rearrange("b (j p) -> b j p", j=JL)
    sc_b = scale.unsqueeze(2).broadcast_to([B, JL, P])
    nc.vector.tensor_mul(vv[:], vv[:], sc_b[:])


def _build_vbd(nc, small, psum_t, v_sb, identity):
    """v_sb [64, (j,p)] fp32 -> two block-diag bf16 lhsT [128, (jj 2, b 64)]."""
    vt_ps = psum_t.tile([128, B], F32, tag="vt_ps")
    nc.tensor.transpose(vt_ps[:], v_sb[:], identity[:B, :B])
    vt = small.tile([128, B], F32, tag="vt")
    nc.scalar.copy(vt[:], vt_ps[:])  # [(j,p), b]
    vbds = []
    for pair in range(2):
        vbd = small.tile([128, 2 * B], BF16, tag=f"vbd{pair}")
        nc.vector.memset(vbd[:], 0.0)
        for jj in range(2):
            j = pair * 2 + jj
            nc.vector.tensor_copy(
                vbd[j * P:(j + 1) * P, jj * B:(jj + 1) * B],
                vt[j * P:(j + 1) * P, :],
            )
        vbds.append(vbd)
    return vbds


def _vT_to_v(nc, small, ps_vt, vT_ps, identity, scale=None):
    """vT psum [128 (j,p), 64 b] -> v_sb [64, (j,p)] fp32 via evac+transpose."""
    vT = small.tile([128, B], F32, tag="vTe")
    if scale is None:
        nc.scalar.copy(vT[:], vT_ps[:])
    else:
        nc.scalar.mul(vT[:], vT_ps[:], scale)
    v_ps = ps_vt.tile([B, 128], F32, tag="v_ps2")
    nc.tensor.transpose(v_ps[:], vT[:], identity[:])
    v_sb = small.tile([B, JL * P], F32, tag="v")
    nc.scalar.copy(v_sb[:], v_ps[:])
    return v_sb


def build_kernel():
    if "nc" in _CACHED:
        return _CACHED["nc"]
    nc = bacc.Bacc(
        "TRN2", target_bir_lowering=False, debug=False, num_devices=N_CORES
    )
    wn_d = nc.dram_tensor("wn", [128, ICH * Q * JL * P], BF16, kind="ExternalInput")
    wt_d = nc.dram_tensor("wt", [128, I * Q], BF16, kind="ExternalInput")
    xq_d = nc.dram_tensor("xq", [128, ICH * Q * B], BF16, kind="ExternalInput")
    xt_d = nc.dram_tensor("xt", [128, I * Q], BF16, kind="ExternalInput")
    out_d = nc.dram_tensor("o", [B, JL * P], F32, kind="ExternalOutput")

    with tile.TileContext(nc) as tc:
        with (
            tc.tile_pool(name="big", bufs=1) as big,
            tc.tile_pool(name="wts", bufs=3) as wts,
            tc.tile_pool(name="tstr", bufs=4) as tstr,
            tc.tile_pool(name="estr", bufs=2) as estr,
            tc.tile_pool(name="small", bufs=1) as small,
            tc.tile_pool(name="ytile", bufs=5) as ytile,
            tc.tile_pool(name="dram", bufs=4, space="DRAM") as dram,
        ):
            # ---- resident loads (chunked so consumers start early) ----
            wn = big.tile([128, ICH * Q * JL * P], BF16, tag="wn")   # 64K/part
            wn_chunk = ICH * Q * JL * P // 4
            for c in range(4):
                nc.sync.dma_start(
                    wn[:, c * wn_chunk:(c + 1) * wn_chunk],
                    wn_d[:, c * wn_chunk:(c + 1) * wn_chunk],
                )
            wnv = wn.rearrange("k (ich q j p) -> k ich q j p", ich=ICH, q=Q, j=JL)
            xq = big.tile([128, ICH * Q * B], BF16, tag="xq")        # 32K/part
            xq_chunk = ICH * Q * B // 2
            for c in range(2):
                nc.sync.dma_start(
                    xq[:, c * xq_chunk:(c + 1) * xq_chunk],
                    xq_d[:, c * xq_chunk:(c + 1) * xq_chunk],
                )
            xqv = xq.rearrange("k (ich q b) -> k ich q b", ich=ICH, q=Q)
            xt = big.tile([128, I * Q], BF16, tag="xt")              # 64K/part

            identity = big.tile([128, 128], F32, tag="ident")
            make_identity(nc, identity[:])
            identb = big.tile([128, 128], BF16, tag="identb")
            make_identity(nc, identb[:])
            eps_t = big.tile([128, 1], F32, tag="eps")
            nc.vector.memset(eps_t[:], EPS)

            # F[ip, (ich, j, b)] bf16: running c (up to global normalizer)
            f_sb = big.tile([128, ICH * JL * B], BF16, tag="f")      # 8K/part
            f_v = f_sb.rearrange("k (ich j b) -> k ich j b", ich=ICH, j=JL)

            # warmup collective to absorb core-start skew
            wu_s = small.tile([128, 8], F32, tag="wu")
            nc.gpsimd.memset(wu_s[:], 0.0)
            wu_i = dram.tile([128, 8], F32, tag="wu_i")
            wu_o = dram.tile([128, 8], F32, tag="wu_o")
            nc.gpsimd.dma_start(wu_i[:], wu_s[:])
            nc.gpsimd.collective_compute(
                "AllReduce", mybir.AluOpType.add,
                replica_groups=[list(range(N_CORES))],
                ins=[wu_i.opt()], outs=[wu_o.opt()],
            )

            # ---- S-pass: vT[(j,p), b] = sum_{i,q} W x ---------------
            with tc.tile_pool(name="ps_s", bufs=1, space="PSUM") as ps_s, \
                 tc.tile_pool(name="ps_st", bufs=1, space="PSUM") as ps_st:
                s_ps = ps_s.tile([128, B], F32, tag="s_ps")
                n_mm = ICH * Q
                k = 0
                for ich in range(ICH):
                    for q in range(Q):
                        nc.tensor.matmul(
                            s_ps[:],
                            wnv[:, ich, q, :, :],       # lhsT [128, (j p)]
                            xqv[:, ich, q, :],          # rhs  [128, 64]
                            start=(k == 0), stop=(k == n_mm - 1),
                        )
                        k += 1
                v_sb = _vT_to_v(nc, small, ps_st, s_ps, identity, scale=1.0 / J)
                _squash(nc, small, v_sb, eps_t)
                vbds = _build_vbd(nc, small, ps_st, v_sb, identity)

            # xt load issued after the S-pass so wn/xq own the early DMA bw
            xt_chunk = I * Q // 4
            for c in range(4):
                nc.sync.dma_start(
                    xt[:, c * xt_chunk:(c + 1) * xt_chunk],
                    xt_d[:, c * xt_chunk:(c + 1) * xt_chunk],
                )

            # ---- 2 routing iterations -------------------------------
            for it in range(2):
                first = it == 0
                # b-pass: Delta[b,j,i] via t = vbd^T Wt, u = t*x, tree over q
                cc_pend = [None, None]
                with tc.tile_pool(name=f"ps_b{it}", bufs=3, space="PSUM") as ps_b, \
                     tc.tile_pool(name=f"ps_bt{it}", bufs=2, space="PSUM") as ps_bt:
                    for g in range(ICH):
                        wt_s = wts.tile([128, 128 * Q], BF16, tag="wt_s")
                        nc.sync.dma_start(
                            wt_s[:], wt_d[:, g * 128 * Q:(g + 1) * 128 * Q]
                        )
                        xoff = g * 2048
                        for pair in range(2):
                            d_ps = ps_bt.tile(
                                [128, 128], BF16, tag="d_ps",
                                name=f"d_ps{it}_{g}_{pair}",
                            )
                            tss = []
                            for half in range(2):
                                t_ps = ps_b.tile(
                                    [128, 1024], F32, tag="t_ps",
                                    name=f"t_ps{it}_{g}_{pair}_{half}",
                                )
                                for m in range(2):
                                    off = half * 1024 + m * 512
                                    nc.tensor.matmul(
                                        t_ps[:, m * 512:(m + 1) * 512],
                                        vbds[pair][:],
                                        wt_s[:, off:off + 512],
                                        start=True, stop=True,
                                    )
                                ts = tstr.tile(
                                    [128, 1024], BF16, tag="ts",
                                    name=f"ts{it}_{g}_{pair}_{half}",
                                )
                                xsl = xt[:, xoff + half * 1024:
                                         xoff + half * 1024 + 1024]
                                hidx = g * 4 + pair * 2 + half
                                if hidx % EVAC_MOD != EVAC_MOD - 1:
                                    nc.scalar.copy(ts[:], t_ps[:])
                                    nc.vector.tensor_mul(ts[:], ts[:], xsl)
                                else:
                                    nc.vector.tensor_mul(ts[:], t_ps[:], xsl)
                                # tree over q within the half: 1024->512->256
                                nc.vector.tensor_add(
                                    ts[:, 0:512], ts[:, 0:512], ts[:, 512:1024]
                                )
                                eng2 = nc.gpsimd if GPSIMD_TREE2 else nc.vector
                                eng2.tensor_add(
                                    ts[:, 0:256], ts[:, 0:256], ts[:, 256:512]
                                )
                                tss.append(ts)
                            engm = nc.gpsimd if GPSIMD_MERGE else nc.vector
                            engm.tensor_add(
                                tss[0][:, 0:256], tss[0][:, 0:256],
                                tss[1][:, 0:256],
                            )
                            nc.vector.tensor_add(
                                tss[0][:, 0:128], tss[0][:, 0:128],
                                tss[0][:, 128:256],
                            )
                            nc.tensor.transpose(
                                d_ps[:], tss[0][:, 0:128], identb[:]
                            )
                            off = (g * JL + pair * 2) * B
                            dst = f_sb[:, off:off + 2 * B]
                            if first:
                                nc.scalar.activation(dst, d_ps[:], AF.Exp)
                            else:
                                ex = estr.tile(
                                    [128, 128], BF16, tag="ex",
                                    name=f"ex{it}_{g}_{pair}",
                                )
                                nc.scalar.activation(ex[:], d_ps[:], AF.Exp)
                                nc.vector.tensor_mul(dst, dst, ex[:])
                        if g == 7 or g == ICH - 1:
                            h = 0 if g == 7 else 1
                            sl = slice(h * 8, h * 8 + 8)
                            zph = small.tile(
                                [128, 8 * B], BF16, tag=f"zp{h}",
                                name=f"zp{it}_{h}",
                            )
                            zpv = zph.rearrange("k (ic b) -> k ic b", ic=8)
                            nc.vector.tensor_add(
                                zpv[:], f_v[:, sl, 0, :], f_v[:, sl, 1, :]
                            )
                            for j in range(2, JL):
                                nc.vector.tensor_add(
                                    zpv[:], zpv[:], f_v[:, sl, j, :]
                                )
                            cc_i = dram.tile(
                                [128, 8 * B], BF16, tag=f"cc_i{h}",
                                name=f"cci{it}_{h}",
                            )
                            cc_o = dram.tile(
                                [128, 8 * B], BF16, tag=f"cc_o{h}",
                                name=f"cco{it}_{h}",
                            )
                            nc.gpsimd.dma_start(cc_i[:], zph[:])
                            nc.gpsimd.collective_compute(
                                "AllReduce", mybir.AluOpType.add,
                                replica_groups=[list(range(N_CORES))],
                                ins=[cc_i.opt()], outs=[cc_o.opt()],
                            )
                            cc_pend[h] = cc_o

                # v-pass: vT[(j,p), b] = sum_{i,q} W (F x), col-tiled over j
                with tc.tile_pool(name=f"ps_v{it}", bufs=1, space="PSUM") as ps_v, \
                     tc.tile_pool(name=f"ps_vt{it}", bufs=2, space="PSUM") as ps_vt:
                    vT_ps = ps_v.tile([128, B], F32, tag="vT_ps")
                    for h in range(2):
                        sl = slice(h * 8, h * 8 + 8)
                        zh = small.tile(
                            [128, 8 * B], BF16, tag=f"z{h}", name=f"z{it}_{h}"
                        )
                        nc.sync.dma_start(zh[:], cc_pend[h][:])
                        with nc.allow_low_precision(
                            reason="softmax normalizer; tol 2e-2"
                        ):
                            nc.vector.reciprocal(zh[:], zh[:])
                        zrv = zh.rearrange("k (ic b) -> k ic b", ic=8)
                        for j in range(JL):
                            nc.vector.tensor_mul(
                                f_v[:, sl, j, :], f_v[:, sl, j, :], zrv[:]
                            )
                    for ich in range(ICH):
                        ys = []
                        for j in range(JL):
                            y = ytile.tile(
                                [128, Q * B], BF16, tag="y",
                                name=f"y{it}_{ich}_{j}",
                            )
                            yv = y.rearrange("k (q b) -> k q b", q=Q)
                            cb = (
                                f_v[:, ich, j, :]
                                .unsqueeze(1).broadcast_to([128, Q, B])
                            )
                            yeng = (
                                nc.gpsimd
                                if (GPSIMD_Y and j == JL - 1) else nc.vector
                            )
                            yeng.tensor_mul(
                                yv[:], xqv[:, ich, :, :], cb[:]
                            )
                            ys.append(yv)
                        for q in range(Q):
                            for j in range(JL):
                                nc.tensor.matmul(
                                    vT_ps[j * P:(j + 1) * P, :],
                                    wnv[:, ich, q, j, :],
                                    ys[j][:, q, :],
                                    start=(ich == 0 and q == 0),
                                    stop=(ich == ICH - 1 and q == Q - 1),
                                    tile_position=(0, j * P),
                                )
                    v_sb = _vT_to_v(nc, small, ps_vt, vT_ps, identity)
                    _squash(nc, small, v_sb, eps_t)
                    if it == 0:
                        vbds = _build_vbd(nc, small, ps_vt, v_sb, identity)
                    else:
                        nc.sync.dma_start(out_d[:], v_sb[:])

    nc.compile()
    _CACHED["nc"] = nc
    return nc


def _prep_inputs(inputs_np, W_np):
    x = np.ascontiguousarray(inputs_np)           # [B, I, Q] f32
    W = np.ascontiguousarray(W_np)                # [J, I, P, Q] f32
    xq = (
        x.reshape(B, ICH, 128, Q).transpose(2, 1, 3, 0)
        .astype(NP_BF16).reshape(128, ICH * Q * B)
    )
    # xt cols ordered (g, q, iw): matches wt streaming windows
    xt_base = (
        x.reshape(B, ICH, 128, Q).transpose(0, 1, 3, 2)   # [b, g, q, iw]
        .astype(NP_BF16).reshape(B, I * Q)
    )
    xt = np.concatenate([xt_base, xt_base], axis=0)
    in_maps = []
    for r in range(N_CORES):
        Wr = W[r * JL:(r + 1) * JL]                       # [4, I, P, Q]
        wn = (
            Wr.reshape(JL, ICH, 128, P, Q).transpose(2, 1, 4, 0, 3)
            .astype(NP_BF16).reshape(128, ICH * Q * JL * P)
        )
        wt = (
            Wr.reshape(JL, ICH, 128, P, Q)
            .transpose(0, 3, 1, 4, 2)                     # [j, p, g, q, iw]
            .astype(NP_BF16).reshape(128, I * Q)
        )
        in_maps.append(
            {
                "wn": np.ascontiguousarray(wn),
                "wt": np.ascontiguousarray(wt),
                "xq": np.ascontiguousarray(xq),
                "xt": np.ascontiguousarray(xt),
            }
        )
    return in_maps


def kernel(inputs, W, _trace=False):
    nc = build_kernel()
    in_maps = _prep_inputs(np.asarray(inputs), np.asarray(W))
    res = run_bass_kernel_spmd(nc, in_maps, list(range(N_CORES)), trace=_trace)
    out = np.concatenate(
        [res.results[r]["o"].reshape(B, JL, P) for r in range(N_CORES)], axis=1
    )
    if _trace:
        kernel.last_exec_ns = res.exec_time_ns
        kernel.last_results = res
    return out.astype(np.float32)


# revision 14
# speedup vs baseline: 1.3802x; 1.0209x over previous
"""DigitCaps dynamic-routing kernel for 8 Trainium2 NeuronCores.

Sharding: J (num_capsule=32) split 8 ways -> 4 capsules per core, batch
replicated. W is SBUF-resident in its natural layout for the i-contraction
GEMMs; the transposed layout is streamed for the p-contraction routing
matmuls. The routing softmax over J uses a cross-core AllReduce of
per-(b,i) partial exp sums; a renormalization-invariance trick keeps a
single running tensor F (= c, up to a shared normalizer) instead of exp(b).

Per core (j = 4 local capsules, B=64, I=2048, Q=16, P=32):
  hat[b,j,i,p] = sum_q x[b,i,q] W[j,i,p,q]       (never materialized)
  v1 = squash(S/32),  S = sum_{i,q} x W          (c1 uniform)
  Delta_k[b,j,i] = sum_q x[b,i,q] * (Wt^T vbd_k)[b,j,(i,q)]
  F <- F * exp(Delta);  Z' = AllReduce_j(sum_j F);  F <- F / Z'   (= c)
  v_k = squash(sum_{i,q} (F x) W)
  out = v3
"""

import numpy as np
import ml_dtypes

import concourse.bacc as bacc
import concourse.mybir as mybir
import concourse.tile as tile
from concourse.bass_utils import run_bass_kernel_spmd
from concourse.masks import make_identity

BF16 = mybir.dt.bfloat16
F32 = mybir.dt.float32
NP_BF16 = ml_dtypes.bfloat16

N_CORES = 8
B = 64
I = 2048
Q = 16
J = 32
P = 32
JL = J // N_CORES
ICH = I // 128
EPS = 1e-7
AF = mybir.ActivationFunctionType

# b-pass half-chunks whose PSUM evac goes through ScalarE (the rest are
# multiplied straight out of PSUM at DVE 1x). Evac unless idx % 8 == 7:
EVAC_MOD = 8
# offload small tree adds to GpSimd (shares an SBUF port with DVE —
# empirical win/loss):
GPSIMD_TREE2 = False
GPSIMD_MERGE = False
GPSIMD_Y = 0  # gpsimd elementwise measured 3x slower + slows DVE via
# shared-SBUF-port contention; keep off

_CACHED = {}


def _squash(nc, small, v_sb, eps_ap):
    """In-place squash over p of v_sb [64, JL*P] fp32 (free = (j, p))."""
    sq = small.tile([B, JL * P], F32, tag="sq")
    nc.vector.tensor_mul(sq[:], v_sb[:], v_sb[:])
    red = sq.rearrange("b (j p) -> b j p", j=JL)
    w = P
    while w > 1:
        h = w // 2
        nc.vector.tensor_add(red[:, :, 0:h], red[:, :, 0:h], red[:, :, h:w])
        w = h
    s2 = small.tile([B, JL], F32, tag="s2")
    nc.vector.tensor_copy(s2[:], red[:, :, 0])
    rt = small.tile([B, JL], F32, tag="rt")
    nc.scalar.activation(rt[:], s2[:], AF.Sqrt, bias=eps_ap[:B, :])
    den = small.tile([B, JL], F32, tag="den")
    nc.vector.tensor_mul(den[:], s2[:], rt[:])
    nc.vector.tensor_add(den[:], den[:], rt[:])
    rec = small.tile([B, JL], F32, tag="rec")
    nc.vector.reciprocal(rec[:], den[:])
    scale = small.tile([B, JL], F32, tag="scale")
    nc.vector.tensor_mul(scale[:], s2[:], rec[:])
    vv = v_sb.rearrange("b (j p) -> b j p", j=JL)
    sc_b = scale.unsqueeze(2).broadcast_to([B, JL, P])
    nc.vector.tensor_mul(vv[:], vv[:], sc_b[:])


def _build_vbd(nc, small, psum_t, v_sb, identity):
    """v_sb [64, (j,p)] fp32 -> two block-diag bf16 lhsT [128, (jj 2, b 64)]."""
    vt_ps = psum_t.tile([128, B], F32, tag="vt_ps")
    nc.tensor.transpose(vt_ps[:], v_sb[:], identity[:B, :B])
    vt = small.tile([128, B], F32, tag="vt")
    nc.scalar.copy(vt[:], vt_ps[:])  # [(j,p), b]
    vbds = []
    for pair in range(2):
        vbd = small.tile([128, 2 * B], BF16, tag=f"vbd{pair}")
        nc.vector.memset(vbd[:], 0.0)
        for jj in range(2):
            j = pair * 2 + jj
            nc.vector.tensor_copy(
                vbd[j * P:(j + 1) * P, jj * B:(jj + 1) * B],
                vt[j * P:(j + 1) * P, :],
            )
        vbds.append(vbd)
    return vbds


def _vT_to_v(nc, small, ps_vt, vT_ps, identity, scale=None):
    """vT psum [128 (j,p), 64 b] -> v_sb [64, (j,p)] fp32 via evac+transpose."""
    vT = small.tile([128, B], F32, tag="vTe")
    if scale is None:
        nc.scalar.copy(vT[:], vT_ps[:])
    else:
        nc.scalar.mul(vT[:], vT_ps[:], scale)
    v_ps = ps_vt.tile([B, 128], F32, tag="v_ps2")
    nc.tensor.transpose(v_ps[:], vT[:], identity[:])
    v_sb = small.tile([B, JL * P], F32, tag="v")
    nc.scalar.copy(v_sb[:], v_ps[:])
    return v_sb


def build_kernel():
    if "nc" in _CACHED:
        return _CACHED["nc"]
    nc = bacc.Bacc(
        "TRN2", target_bir_lowering=False, debug=False, num_devices=N_CORES
    )
    wn_d = nc.dram_tensor("wn", [128, ICH * Q * JL * P], BF16, kind="ExternalInput")
    wt_d = nc.dram_tensor("wt", [128, I * Q], BF16, kind="ExternalInput")
    xq_d = nc.dram_tensor("xq", [128, ICH * Q * B], BF16, kind="ExternalInput")
    xt_d = nc.dram_tensor("xt", [128, I * Q], BF16, kind="ExternalInput")
    out_d = nc.dram_tensor("o", [B, JL * P], F32, kind="ExternalOutput")

    with tile.TileContext(nc) as tc:
        with (
            tc.tile_pool(name="big", bufs=1) as big,
            tc.tile_pool(name="wts", bufs=3) as wts,
            tc.tile_pool(name="tstr", bufs=4) as tstr,
            tc.tile_pool(name="estr", bufs=2) as estr,
            tc.tile_pool(name="small", bufs=1) as small,
            tc.tile_pool(name="ytile", bufs=5) as ytile,
            tc.tile_pool(name="dram", bufs=4, space="DRAM") as dram,
        ):
            # ---- resident loads (chunked + interleaved so the S-pass
            # starts as soon as its first (wn, xq) chunks land) ----
            wn = big.tile([128, ICH * Q * JL * P], BF16, tag="wn")   # 64K/part
            wnv = wn.rearrange("k (ich q j p) -> k ich q j p", ich=ICH, q=Q, j=JL)
            xq = big.tile([128, ICH * Q * B], BF16, tag="xq")        # 32K/part
            xqv = xq.rearrange("k (ich q b) -> k ich q b", ich=ICH, q=Q)
            wn_chunk = ICH * Q * JL * P // 4
            xq_chunk = ICH * Q * B // 2

            def _wn_load(c):
                nc.sync.dma_start(
                    wn[:, c * wn_chunk:(c + 1) * wn_chunk],
                    wn_d[:, c * wn_chunk:(c + 1) * wn_chunk],
                )

            def _xq_load(c):
                nc.sync.dma_start(
                    xq[:, c * xq_chunk:(c + 1) * xq_chunk],
                    xq_d[:, c * xq_chunk:(c + 1) * xq_chunk],
                )

            _wn_load(0)
            _xq_load(0)
            _wn_load(1)
            _wn_load(2)
            _xq_load(1)
            _wn_load(3)
            xt = big.tile([128, I * Q], BF16, tag="xt")              # 64K/part

            identity = big.tile([128, 128], F32, tag="ident")
            make_identity(nc, identity[:])
            identb = big.tile([128, 128], BF16, tag="identb")
            make_identity(nc, identb[:])
            eps_t = big.tile([128, 1], F32, tag="eps")
            nc.vector.memset(eps_t[:], EPS)

            # F[ip, (ich, j, b)] bf16: running c (up to global normalizer)
            f_sb = big.tile([128, ICH * JL * B], BF16, tag="f")      # 8K/part
            f_v = f_sb.rearrange("k (ich j b) -> k ich j b", ich=ICH, j=JL)

            # warmup collective to absorb core-start skew
            wu_s = small.tile([128, 8], F32, tag="wu")
            nc.gpsimd.memset(wu_s[:], 0.0)
            wu_i = dram.tile([128, 8], F32, tag="wu_i")
            wu_o = dram.tile([128, 8], F32, tag="wu_o")
            nc.gpsimd.dma_start(wu_i[:], wu_s[:])
            nc.gpsimd.collective_compute(
                "AllReduce", mybir.AluOpType.add,
                replica_groups=[list(range(N_CORES))],
                ins=[wu_i.opt()], outs=[wu_o.opt()],
            )

            # ---- S-pass: vT[(j,p), b] = sum_{i,q} W x ---------------
            with tc.tile_pool(name="ps_s", bufs=1, space="PSUM") as ps_s, \
                 tc.tile_pool(name="ps_st", bufs=1, space="PSUM") as ps_st:
                s_ps = ps_s.tile([128, B], F32, tag="s_ps")
                n_mm = ICH * Q
                k = 0
                for ich in range(ICH):
                    for q in range(Q):
                        nc.tensor.matmul(
                            s_ps[:],
                            wnv[:, ich, q, :, :],       # lhsT [128, (j p)]
                            xqv[:, ich, q, :],          # rhs  [128, 64]
                            start=(k == 0), stop=(k == n_mm - 1),
                        )
                        k += 1
                v_sb = _vT_to_v(nc, small, ps_st, s_ps, identity, scale=1.0 / J)
                _squash(nc, small, v_sb, eps_t)
                vbds = _build_vbd(nc, small, ps_st, v_sb, identity)

            # xt load issued after the S-pass so wn/xq own the early DMA bw
            xt_chunk = I * Q // 4
            for c in range(4):
                nc.sync.dma_start(
                    xt[:, c * xt_chunk:(c + 1) * xt_chunk],
                    xt_d[:, c * xt_chunk:(c + 1) * xt_chunk],
                )

            # ---- 2 routing iterations -------------------------------
            for it in range(2):
                first = it == 0
                # b-pass: Delta[b,j,i] via t = vbd^T Wt, u = t*x, tree over q
                cc_pend = [None, None]
                with tc.tile_pool(name=f"ps_b{it}", bufs=3, space="PSUM") as ps_b, \
                     tc.tile_pool(name=f"ps_bt{it}", bufs=2, space="PSUM") as ps_bt:
                    for g in range(ICH):
                        wt_s = wts.tile([128, 128 * Q], BF16, tag="wt_s")
                        nc.sync.dma_start(
                            wt_s[:], wt_d[:, g * 128 * Q:(g + 1) * 128 * Q]
                        )
                        xoff = g * 2048
                        for pair in range(2):
                            # Delta^T accumulates in PSUM f32: the last two
                            # tree levels fold into 4 accumulating
                            # matmul-transposes (lhsT=state, rhs=identity).
                            d_ps = ps_bt.tile(
                                [128, 128], F32, tag="d_ps",
                                name=f"d_ps{it}_{g}_{pair}",
                            )
                            for half in range(2):
                                t_ps = ps_b.tile(
                                    [128, 1024], F32, tag="t_ps",
                                    name=f"t_ps{it}_{g}_{pair}_{half}",
                                )
                                for m in range(2):
                                    off = half * 1024 + m * 512
                                    nc.tensor.matmul(
                                        t_ps[:, m * 512:(m + 1) * 512],
                                        vbds[pair][:],
                                        wt_s[:, off:off + 512],
                                        start=True, stop=True,
                                    )
                                ts = tstr.tile(
                                    [128, 1024], BF16, tag="ts",
                                    name=f"ts{it}_{g}_{pair}_{half}",
                                )
                                xsl = xt[:, xoff + half * 1024:
                                         xoff + half * 1024 + 1024]
                                hidx = g * 4 + pair * 2 + half
                                if hidx % EVAC_MOD != EVAC_MOD - 1:
                                    nc.scalar.copy(ts[:], t_ps[:])
                                    nc.vector.tensor_mul(ts[:], ts[:], xsl)
                                else:
                                    nc.vector.tensor_mul(ts[:], t_ps[:], xsl)
                                # tree over q within the half: 1024->512->256
                                nc.vector.tensor_add(
                                    ts[:, 0:512], ts[:, 0:512], ts[:, 512:1024]
                                )
                                nc.vector.tensor_add(
                                    ts[:, 0:256], ts[:, 0:256], ts[:, 256:512]
                                )
                                for m in range(2):
                                    nc.tensor.matmul(
                                        d_ps[:],
                                        ts[:, m * 128:(m + 1) * 128],
                                        identb[:],
                                        start=(half == 0 and m == 0),
                                        stop=(half == 1 and m == 1),
                                    )
                            off = (g * JL + pair * 2) * B
                            dst = f_sb[:, off:off + 2 * B]
                            if first:
                                nc.scalar.activation(dst, d_ps[:], AF.Exp)
                            else:
                                ex = estr.tile(
                                    [128, 128], BF16, tag="ex",
                                    name=f"ex{it}_{g}_{pair}",
                                )
                                nc.scalar.activation(ex[:], d_ps[:], AF.Exp)
                                nc.vector.tensor_mul(dst, dst, ex[:])
                        if g == 7 or g == ICH - 1:
                            h = 0 if g == 7 else 1
                            sl = slice(h * 8, h * 8 + 8)
                            zph = small.tile(
                                [128, 8 * B], BF16, tag=f"zp{h}",
                                name=f"zp{it}_{h}",
                            )
                            zpv = zph.rearrange("k (ic b) -> k ic b", ic=8)
                            nc.vector.tensor_add(
                                zpv[:], f_v[:, sl, 0, :], f_v[:, sl, 1, :]
                            )
                            for j in range(2, JL):
                                nc.vector.tensor_add(
                                    zpv[:], zpv[:], f_v[:, sl, j, :]
                                )
                            cc_i = dram.tile(
                                [128, 8 * B], BF16, tag=f"cc_i{h}",
                                name=f"cci{it}_{h}",
                            )
                            cc_o = dram.tile(
                                [128, 8 * B], BF16, tag=f"cc_o{h}",
                                name=f"cco{it}_{h}",
                            )
                            nc.gpsimd.dma_start(cc_i[:], zph[:])
                            nc.gpsimd.collective_compute(
                                "AllReduce", mybir.AluOpType.add,
                                replica_groups=[list(range(N_CORES))],
                                ins=[cc_i.opt()], outs=[cc_o.opt()],
                            )
                            cc_pend[h] = cc_o

                # v-pass: vT[(j,p), b] = sum_{i,q} W (F x), col-tiled over j
                with tc.tile_pool(name=f"ps_v{it}", bufs=1, space="PSUM") as ps_v, \
                     tc.tile_pool(name=f"ps_vt{it}", bufs=2, space="PSUM") as ps_vt:
                    vT_ps = ps_v.tile([128, B], F32, tag="vT_ps")
                    for h in range(2):
                        sl = slice(h * 8, h * 8 + 8)
                        zh = small.tile(
                            [128, 8 * B], BF16, tag=f"z{h}", name=f"z{it}_{h}"
                        )
                        nc.sync.dma_start(zh[:], cc_pend[h][:])
                        with nc.allow_low_precision(
                            reason="softmax normalizer; tol 2e-2"
                        ):
                            nc.vector.reciprocal(zh[:], zh[:])
                        zrv = zh.rearrange("k (ic b) -> k ic b", ic=8)
                        for j in range(JL):
                            nc.vector.tensor_mul(
                                f_v[:, sl, j, :], f_v[:, sl, j, :], zrv[:]
                            )
                    for ich in range(ICH):
                        ys = []
                        for j in range(JL):
                            y = ytile.tile(
                                [128, Q * B], BF16, tag="y",
                                name=f"y{it}_{ich}_{j}",
                            )
                            yv = y.rearrange("k (q b) -> k q b", q=Q)
                            cb = (
                                f_v[:, ich, j, :]
                                .unsqueeze(1).broadcast_to([128, Q, B])
                            )
                            yeng = (
                                nc.gpsimd
                                if (GPSIMD_Y and j == JL - 1) else nc.vector
                            )
                            yeng.tensor_mul(
                                yv[:], xqv[:, ich, :, :], cb[:]
                            )
                            ys.append(yv)
                        for q in range(Q):
                            for j in range(JL):
                                nc.tensor.matmul(
                                    vT_ps[j * P:(j + 1) * P, :],
                                    wnv[:, ich, q, j, :],
                                    ys[j][:, q, :],
                                    start=(ich == 0 and q == 0),
                                    stop=(ich == ICH - 1 and q == Q - 1),
                                    tile_position=(0, j * P),
                                )
                    v_sb = _vT_to_v(nc, small, ps_vt, vT_ps, identity)
                    _squash(nc, small, v_sb, eps_t)
                    if it == 0:
                        vbds = _build_vbd(nc, small, ps_vt, v_sb, identity)
                    else:
                        nc.sync.dma_start(out_d[:], v_sb[:])

    nc.compile()
    _CACHED["nc"] = nc
    return nc


def _prep_inputs(inputs_np, W_np):
    x = np.ascontiguousarray(inputs_np)           # [B, I, Q] f32
    W = np.ascontiguousarray(W_np)                # [J, I, P, Q] f32
    xq = (
        x.reshape(B, ICH, 128, Q).transpose(2, 1, 3, 0)
        .astype(NP_BF16).reshape(128, ICH * Q * B)
    )
    # xt cols ordered (g, q, iw): matches wt streaming windows
    xt_base = (
        x.reshape(B, ICH, 128, Q).transpose(0, 1, 3, 2)   # [b, g, q, iw]
        .astype(NP_BF16).reshape(B, I * Q)
    )
    xt = np.concatenate([xt_base, xt_base], axis=0)
    in_maps = []
    for r in range(N_CORES):
        Wr = W[r * JL:(r + 1) * JL]                       # [4, I, P, Q]
        wn = (
            Wr.reshape(JL, ICH, 128, P, Q).transpose(2, 1, 4, 0, 3)
            .astype(NP_BF16).reshape(128, ICH * Q * JL * P)
        )
        wt = (
            Wr.reshape(JL, ICH, 128, P, Q)
            .transpose(0, 3, 1, 4, 2)                     # [j, p, g, q, iw]
            .astype(NP_BF16).reshape(128, I * Q)
        )
        in_maps.append(
            {
                "wn": np.ascontiguousarray(wn),
                "wt": np.ascontiguousarray(wt),
                "xq": np.ascontiguousarray(xq),
                "xt": np.ascontiguousarray(xt),
            }
        )
    return in_maps


def kernel(inputs, W, _trace=False):
    nc = build_kernel()
    in_maps = _prep_inputs(np.asarray(inputs), np.asarray(W))
    res = run_bass_kernel_spmd(nc, in_maps, list(range(N_CORES)), trace=_trace)
    out = np.concatenate(
        [res.results[r]["o"].reshape(B, JL, P) for r in range(N_CORES)], axis=1
    )
    if _trace:
        kernel.last_exec_ns = res.exec_time_ns
        kernel.last_results = res
    return out.astype(np.float32)


# revision 16
# speedup vs baseline: 1.5056x; 1.0908x over previous
"""DigitCaps dynamic-routing kernel for 8 Trainium2 NeuronCores.

Sharding: J (num_capsule=32) split 8 ways -> 4 capsules per core, batch
replicated. W is SBUF-resident in its natural layout for the i-contraction
GEMMs; the transposed layout is streamed for the p-contraction routing
matmuls. The routing softmax over J uses a cross-core AllReduce of
per-(b,i) partial exp sums; a renormalization-invariance trick keeps a
single running tensor F (= c, up to a shared normalizer) instead of exp(b).

Per core (j = 4 local capsules, B=64, I=2048, Q=16, P=32):
  hat[b,j,i,p] = sum_q x[b,i,q] W[j,i,p,q]       (never materialized)
  v1 = squash(S/32),  S = sum_{i,q} x W          (c1 uniform)
  Delta_k[b,j,i] = sum_q x[b,i,q] * (Wt^T vbd_k)[b,j,(i,q)]
  F <- F * exp(Delta);  Z' = AllReduce_j(sum_j F);  F <- F / Z'   (= c)
  v_k = squash(sum_{i,q} (F x) W)
  out = v3
"""

import numpy as np
import ml_dtypes

import concourse.bacc as bacc
import concourse.mybir as mybir
import concourse.tile as tile
from concourse.bass_utils import run_bass_kernel_spmd
from concourse.masks import make_identity

BF16 = mybir.dt.bfloat16
F32 = mybir.dt.float32
NP_BF16 = ml_dtypes.bfloat16

N_CORES = 8
B = 64
I = 2048
Q = 16
J = 32
P = 32
JL = J // N_CORES
ICH = I // 128
EPS = 1e-7
AF = mybir.ActivationFunctionType

# b-pass half-chunks whose PSUM evac goes through ScalarE (the rest are
# multiplied straight out of PSUM at DVE 1x). Evac unless idx % 8 == 7:
EVAC_MOD = 8
# offload small tree adds to GpSimd (shares an SBUF port with DVE —
# empirical win/loss):
GPSIMD_TREE2 = False
GPSIMD_MERGE = False
GPSIMD_Y = 0  # gpsimd elementwise measured 3x slower + slows DVE via
# shared-SBUF-port contention; keep off

_CACHED = {}


def _squash(nc, small, v_sb, eps_ap):
    """In-place squash over p of v_sb [64, JL*P] fp32 (free = (j, p))."""
    sq = small.tile([B, JL * P], F32, tag="sq")
    nc.vector.tensor_mul(sq[:], v_sb[:], v_sb[:])
    red = sq.rearrange("b (j p) -> b j p", j=JL)
    w = P
    while w > 1:
        h = w // 2
        nc.vector.tensor_add(red[:, :, 0:h], red[:, :, 0:h], red[:, :, h:w])
        w = h
    s2 = small.tile([B, JL], F32, tag="s2")
    nc.vector.tensor_copy(s2[:], red[:, :, 0])
    rt = small.tile([B, JL], F32, tag="rt")
    nc.scalar.activation(rt[:], s2[:], AF.Sqrt, bias=eps_ap[:B, :])
    den = small.tile([B, JL], F32, tag="den")
    nc.vector.tensor_mul(den[:], s2[:], rt[:])
    nc.vector.tensor_add(den[:], den[:], rt[:])
    rec = small.tile([B, JL], F32, tag="rec")
    nc.vector.reciprocal(rec[:], den[:])
    scale = small.tile([B, JL], F32, tag="scale")
    nc.vector.tensor_mul(scale[:], s2[:], rec[:])
    vv = v_sb.rearrange("b (j p) -> b j p", j=JL)
    sc_b = scale.unsqueeze(2).broadcast_to([B, JL, P])
    nc.vector.tensor_mul(vv[:], vv[:], sc_b[:])


def _build_vbd(nc, small, psum_t, v_sb, identity):
    """v_sb [64, (j,p)] fp32 -> two block-diag bf16 lhsT [128, (jj 2, b 64)]."""
    vt_ps = psum_t.tile([128, B], F32, tag="vt_ps")
    nc.tensor.transpose(vt_ps[:], v_sb[:], identity[:B, :B])
    vt = small.tile([128, B], F32, tag="vt")
    nc.scalar.copy(vt[:], vt_ps[:])  # [(j,p), b]
    vbds = []
    for pair in range(2):
        vbd = small.tile([128, 2 * B], BF16, tag=f"vbd{pair}")
        nc.vector.memset(vbd[:], 0.0)
        for jj in range(2):
            j = pair * 2 + jj
            nc.vector.tensor_copy(
                vbd[j * P:(j + 1) * P, jj * B:(jj + 1) * B],
                vt[j * P:(j + 1) * P, :],
            )
        vbds.append(vbd)
    return vbds


def _vT_to_v(nc, small, ps_vt, vT_ps, identity, scale=None):
    """vT psum [128 (j,p), 64 b] -> v_sb [64, (j,p)] fp32 via evac+transpose."""
    vT = small.tile([128, B], F32, tag="vTe")
    if scale is None:
        nc.scalar.copy(vT[:], vT_ps[:])
    else:
        nc.scalar.mul(vT[:], vT_ps[:], scale)
    v_ps = ps_vt.tile([B, 128], F32, tag="v_ps2")
    nc.tensor.transpose(v_ps[:], vT[:], identity[:])
    v_sb = small.tile([B, JL * P], F32, tag="v")
    nc.scalar.copy(v_sb[:], v_ps[:])
    return v_sb


def build_kernel():
    if "nc" in _CACHED:
        return _CACHED["nc"]
    nc = bacc.Bacc(
        "TRN2", target_bir_lowering=False, debug=False, num_devices=N_CORES
    )
    wn_d = nc.dram_tensor("wn", [128, ICH * Q * JL * P], BF16, kind="ExternalInput")
    wt_d = nc.dram_tensor("wt", [128, I * Q], BF16, kind="ExternalInput")
    xq_d = nc.dram_tensor("xq", [128, ICH * Q * B], BF16, kind="ExternalInput")
    xt_d = nc.dram_tensor("xt", [128, I * Q], BF16, kind="ExternalInput")
    out_d = nc.dram_tensor("o", [B, JL * P], F32, kind="ExternalOutput")

    with tile.TileContext(nc) as tc:
        with (
            tc.tile_pool(name="big", bufs=1) as big,
            tc.tile_pool(name="wts", bufs=3) as wts,
            tc.tile_pool(name="tstr", bufs=4) as tstr,
            tc.tile_pool(name="estr", bufs=2) as estr,
            tc.tile_pool(name="small", bufs=1) as small,
            tc.tile_pool(name="ytile", bufs=5) as ytile,
            tc.tile_pool(name="dram", bufs=4, space="DRAM") as dram,
        ):
            # ---- resident loads (chunked + interleaved so the S-pass
            # starts as soon as its first (wn, xq) chunks land) ----
            wn = big.tile([128, ICH * Q * JL * P], BF16, tag="wn")   # 64K/part
            wnv = wn.rearrange("k (ich q j p) -> k ich q j p", ich=ICH, q=Q, j=JL)
            xq = big.tile([128, ICH * Q * B], BF16, tag="xq")        # 32K/part
            xqv = xq.rearrange("k (ich q b) -> k ich q b", ich=ICH, q=Q)
            wn_chunk = ICH * Q * JL * P // 4
            xq_chunk = ICH * Q * B // 2

            def _wn_load(c):
                nc.sync.dma_start(
                    wn[:, c * wn_chunk:(c + 1) * wn_chunk],
                    wn_d[:, c * wn_chunk:(c + 1) * wn_chunk],
                )

            def _xq_load(c):
                nc.sync.dma_start(
                    xq[:, c * xq_chunk:(c + 1) * xq_chunk],
                    xq_d[:, c * xq_chunk:(c + 1) * xq_chunk],
                )

            _wn_load(0)
            _xq_load(0)
            _wn_load(1)
            _wn_load(2)
            _xq_load(1)
            _wn_load(3)
            xt = big.tile([128, I * Q], BF16, tag="xt")              # 64K/part

            identity = big.tile([128, 128], F32, tag="ident")
            make_identity(nc, identity[:])
            identb = big.tile([128, 128], BF16, tag="identb")
            make_identity(nc, identb[:])
            eps_t = big.tile([128, 1], F32, tag="eps")
            nc.vector.memset(eps_t[:], EPS)

            # F[ip, (ich, j, b)] bf16: running c (up to global normalizer)
            f_sb = big.tile([128, ICH * JL * B], BF16, tag="f")      # 8K/part
            f_v = f_sb.rearrange("k (ich j b) -> k ich j b", ich=ICH, j=JL)

            # warmup collective to absorb core-start skew
            wu_s = small.tile([128, 8], F32, tag="wu")
            nc.gpsimd.memset(wu_s[:], 0.0)
            wu_i = dram.tile([128, 8], F32, tag="wu_i")
            wu_o = dram.tile([128, 8], F32, tag="wu_o")
            nc.gpsimd.dma_start(wu_i[:], wu_s[:])
            nc.gpsimd.collective_compute(
                "AllReduce", mybir.AluOpType.add,
                replica_groups=[list(range(N_CORES))],
                ins=[wu_i.opt()], outs=[wu_o.opt()],
            )

            # ---- S-pass: vT[(j,p), b] = sum_{i,q} W x ---------------
            with tc.tile_pool(name="ps_s", bufs=1, space="PSUM") as ps_s, \
                 tc.tile_pool(name="ps_st", bufs=1, space="PSUM") as ps_st:
                s_ps = ps_s.tile([128, B], F32, tag="s_ps")
                n_mm = ICH * Q
                k = 0
                for ich in range(ICH):
                    for q in range(Q):
                        nc.tensor.matmul(
                            s_ps[:],
                            wnv[:, ich, q, :, :],       # lhsT [128, (j p)]
                            xqv[:, ich, q, :],          # rhs  [128, 64]
                            start=(k == 0), stop=(k == n_mm - 1),
                        )
                        k += 1
                v_sb = _vT_to_v(nc, small, ps_st, s_ps, identity, scale=1.0 / J)
                _squash(nc, small, v_sb, eps_t)
                vbds = _build_vbd(nc, small, ps_st, v_sb, identity)

            # xt load gated behind the wn tail (tiny WAW sliver dep) so the
            # S-pass owns the early DMA bandwidth; b-pass needs xt ~20us
            # later than wn anyway.
            xt_chunk = I * Q // 4
            for c in range(4):
                nc.vector.tensor_copy(
                    xt[:, c * xt_chunk:c * xt_chunk + 1], wn[:, -1:]
                )
                nc.sync.dma_start(
                    xt[:, c * xt_chunk:(c + 1) * xt_chunk],
                    xt_d[:, c * xt_chunk:(c + 1) * xt_chunk],
                )

            # ---- 2 routing iterations -------------------------------
            for it in range(2):
                first = it == 0
                # b-pass: Delta[b,j,i] via t = vbd^T Wt, u = t*x, tree over q
                cc_pend = [None, None]
                with tc.tile_pool(name=f"ps_b{it}", bufs=3, space="PSUM") as ps_b, \
                     tc.tile_pool(name=f"ps_bt{it}", bufs=2, space="PSUM") as ps_bt:
                    for g in range(ICH):
                        wt_s = wts.tile([128, 128 * Q], BF16, tag="wt_s")
                        nc.sync.dma_start(
                            wt_s[:], wt_d[:, g * 128 * Q:(g + 1) * 128 * Q]
                        )
                        xoff = g * 2048
                        for pair in range(2):
                            # Delta^T accumulates in PSUM f32: the last two
                            # tree levels fold into 4 accumulating
                            # matmul-transposes (lhsT=state, rhs=identity).
                            d_ps = ps_bt.tile(
                                [128, 128], F32, tag="d_ps",
                                name=f"d_ps{it}_{g}_{pair}",
                            )
                            for half in range(2):
                                t_ps = ps_b.tile(
                                    [128, 1024], F32, tag="t_ps",
                                    name=f"t_ps{it}_{g}_{pair}_{half}",
                                )
                                for m in range(2):
                                    off = half * 1024 + m * 512
                                    nc.tensor.matmul(
                                        t_ps[:, m * 512:(m + 1) * 512],
                                        vbds[pair][:],
                                        wt_s[:, off:off + 512],
                                        start=True, stop=True,
                                    )
                                ts = tstr.tile(
                                    [128, 1024], BF16, tag="ts",
                                    name=f"ts{it}_{g}_{pair}_{half}",
                                )
                                xsl = xt[:, xoff + half * 1024:
                                         xoff + half * 1024 + 1024]
                                hidx = g * 4 + pair * 2 + half
                                if hidx % EVAC_MOD != EVAC_MOD - 1:
                                    nc.scalar.copy(ts[:], t_ps[:])
                                    nc.vector.tensor_mul(ts[:], ts[:], xsl)
                                else:
                                    nc.vector.tensor_mul(ts[:], t_ps[:], xsl)
                                # tree over q within the half: 1024->512;
                                # remaining levels fold into accumulating
                                # PE matmul-transposes
                                nc.vector.tensor_add(
                                    ts[:, 0:512], ts[:, 0:512], ts[:, 512:1024]
                                )
                                for m in range(4):
                                    nc.tensor.matmul(
                                        d_ps[:],
                                        ts[:, m * 128:(m + 1) * 128],
                                        identb[:],
                                        start=(half == 0 and m == 0),
                                        stop=(half == 1 and m == 3),
                                    )
                            off = (g * JL + pair * 2) * B
                            dst = f_sb[:, off:off + 2 * B]
                            if first:
                                nc.scalar.activation(dst, d_ps[:], AF.Exp)
                            else:
                                ex = estr.tile(
                                    [128, 128], BF16, tag="ex",
                                    name=f"ex{it}_{g}_{pair}",
                                )
                                nc.scalar.activation(ex[:], d_ps[:], AF.Exp)
                                nc.vector.tensor_mul(dst, dst, ex[:])
                        if g == 7 or g == ICH - 1:
                            h = 0 if g == 7 else 1
                            sl = slice(h * 8, h * 8 + 8)
                            zph = small.tile(
                                [128, 8 * B], BF16, tag=f"zp{h}",
                                name=f"zp{it}_{h}",
                            )
                            zpv = zph.rearrange("k (ic b) -> k ic b", ic=8)
                            nc.vector.tensor_add(
                                zpv[:], f_v[:, sl, 0, :], f_v[:, sl, 1, :]
                            )
                            for j in range(2, JL):
                                nc.vector.tensor_add(
                                    zpv[:], zpv[:], f_v[:, sl, j, :]
                                )
                            cc_i = dram.tile(
                                [128, 8 * B], BF16, tag=f"cc_i{h}",
                                name=f"cci{it}_{h}",
                            )
                            cc_o = dram.tile(
                                [128, 8 * B], BF16, tag=f"cc_o{h}",
                                name=f"cco{it}_{h}",
                            )
                            nc.gpsimd.dma_start(cc_i[:], zph[:])
                            nc.gpsimd.collective_compute(
                                "AllReduce", mybir.AluOpType.add,
                                replica_groups=[list(range(N_CORES))],
                                ins=[cc_i.opt()], outs=[cc_o.opt()],
                            )
                            cc_pend[h] = cc_o

                # v-pass: vT[(j,p), b] = sum_{i,q} W (F x), col-tiled over j
                with tc.tile_pool(name=f"ps_v{it}", bufs=1, space="PSUM") as ps_v, \
                     tc.tile_pool(name=f"ps_vt{it}", bufs=2, space="PSUM") as ps_vt:
                    vT_ps = ps_v.tile([128, B], F32, tag="vT_ps")
                    for h in range(2):
                        sl = slice(h * 8, h * 8 + 8)
                        zh = small.tile(
                            [128, 8 * B], BF16, tag=f"z{h}", name=f"z{it}_{h}"
                        )
                        nc.sync.dma_start(zh[:], cc_pend[h][:])
                        with nc.allow_low_precision(
                            reason="softmax normalizer; tol 2e-2"
                        ):
                            nc.vector.reciprocal(zh[:], zh[:])
                        zrv = zh.rearrange("k (ic b) -> k ic b", ic=8)
                        for j in range(JL):
                            nc.vector.tensor_mul(
                                f_v[:, sl, j, :], f_v[:, sl, j, :], zrv[:]
                            )
                    for ich in range(ICH):
                        ys = []
                        for j in range(JL):
                            y = ytile.tile(
                                [128, Q * B], BF16, tag="y",
                                name=f"y{it}_{ich}_{j}",
                            )
                            yv = y.rearrange("k (q b) -> k q b", q=Q)
                            cb = (
                                f_v[:, ich, j, :]
                                .unsqueeze(1).broadcast_to([128, Q, B])
                            )
                            yeng = (
                                nc.gpsimd
                                if (GPSIMD_Y and j == JL - 1) else nc.vector
                            )
                            yeng.tensor_mul(
                                yv[:], xqv[:, ich, :, :], cb[:]
                            )
                            ys.append(yv)
                        for q in range(Q):
                            for j in range(JL):
                                nc.tensor.matmul(
                                    vT_ps[j * P:(j + 1) * P, :],
                                    wnv[:, ich, q, j, :],
                                    ys[j][:, q, :],
                                    start=(ich == 0 and q == 0),
                                    stop=(ich == ICH - 1 and q == Q - 1),
                                    tile_position=(0, j * P),
                                )
                    v_sb = _vT_to_v(nc, small, ps_vt, vT_ps, identity)
                    _squash(nc, small, v_sb, eps_t)
                    if it == 0:
                        vbds = _build_vbd(nc, small, ps_vt, v_sb, identity)
                    else:
                        nc.sync.dma_start(out_d[:], v_sb[:])

    nc.compile()
    _CACHED["nc"] = nc
    return nc


def _prep_inputs(inputs_np, W_np):
    x = np.ascontiguousarray(inputs_np)           # [B, I, Q] f32
    W = np.ascontiguousarray(W_np)                # [J, I, P, Q] f32
    xq = (
        x.reshape(B, ICH, 128, Q).transpose(2, 1, 3, 0)
        .astype(NP_BF16).reshape(128, ICH * Q * B)
    )
    # xt cols ordered (g, q, iw): matches wt streaming windows
    xt_base = (
        x.reshape(B, ICH, 128, Q).transpose(0, 1, 3, 2)   # [b, g, q, iw]
        .astype(NP_BF16).reshape(B, I * Q)
    )
    xt = np.concatenate([xt_base, xt_base], axis=0)
    in_maps = []
    for r in range(N_CORES):
        Wr = W[r * JL:(r + 1) * JL]                       # [4, I, P, Q]
        wn = (
            Wr.reshape(JL, ICH, 128, P, Q).transpose(2, 1, 4, 0, 3)
            .astype(NP_BF16).reshape(128, ICH * Q * JL * P)
        )
        wt = (
            Wr.reshape(JL, ICH, 128, P, Q)
            .transpose(0, 3, 1, 4, 2)                     # [j, p, g, q, iw]
            .astype(NP_BF16).reshape(128, I * Q)
        )
        in_maps.append(
            {
                "wn": np.ascontiguousarray(wn),
                "wt": np.ascontiguousarray(wt),
                "xq": np.ascontiguousarray(xq),
                "xt": np.ascontiguousarray(xt),
            }
        )
    return in_maps


def kernel(inputs, W, _trace=False):
    nc = build_kernel()
    in_maps = _prep_inputs(np.asarray(inputs), np.asarray(W))
    res = run_bass_kernel_spmd(nc, in_maps, list(range(N_CORES)), trace=_trace)
    out = np.concatenate(
        [res.results[r]["o"].reshape(B, JL, P) for r in range(N_CORES)], axis=1
    )
    if _trace:
        kernel.last_exec_ns = res.exec_time_ns
        kernel.last_results = res
    return out.astype(np.float32)
